# revision 12
# baseline (speedup 1.0000x reference)
"""Multi-headed attention TRN2 Bass kernel.

Problem: B=2, S=2048, D=1024, H=16 heads (dh=64), fp32, bool mask.

Sharding (8 cores): data-parallel over B (2) x tensor-parallel over heads
(4 heads / 256 features per core). Each core computes its head-group's
q/k/v projections, masked softmax attention, and a partial output
projection (Wo columns for its heads). Host sums the 4 partials per batch
element (the TP all-reduce) and adds the bias.

Per-core kernel design (all layouts transposed, i.e. feature-major):
  phase 1: qT/kT pair tiles [128 feat, S] and v tiles [128 s, 256 feat]
           via fp32r matmuls from xT [D, S].
  phase 2: scores_T[k, q] per head-pair via row-packed K=64 fp32r matmuls
           (head a on partitions 0:64, head b on 64:128, concurrent on PE).
  phase 3: exp on ACT (PSUM->fp16 SBUF), multiplicative mask on DVE
           (keep mask, fp16, broadcast over the 2 packed heads), then
           ctx_T accumulation (fp16 matmuls, col-packed pair) plus
           denominators via all-ones matmuls. Softmax normalization by
           reciprocal-multiply at PSUM eviction.
  phase 4: partial outT [D, S] = WoT.T @ ctx_T via fp32r matmuls.

No max-subtraction in softmax: scores are ~N(0,1) (|s| < ~7), exp is
computed in fp32->fp16 which is exact enough (validated 4e-4 rel err
end to end vs the fp32 reference).
"""

import math
from contextlib import ExitStack

import numpy as np

import concourse.mybir as mybir
import concourse.tile as tile
from concourse import bacc
from concourse.bass_utils import run_bass_kernel_spmd

B, S, D, H = 2, 2048, 1024, 16
DH = D // H                 # 64
NCORES = 8
GROUPS = NCORES // B        # 4 head-groups per batch element
FPC = D // GROUPS           # 256 features (4 heads) per core
P = 128
SC = 512                    # q/s chunk (free dim of most matmuls)
NQC = S // SC               # 4
NKT = S // P                # 16 k-position tiles
NDT = D // P                # 8 contraction tiles over D

F32 = mybir.dt.float32
F32R = mybir.dt.float32r
F16 = mybir.dt.float16

EXP = mybir.ActivationFunctionType.Exp
MULT = mybir.AluOpType.mult


def _r(ap):
    return ap.bitcast(F32R)


# dev bisection knob: "full", "dma", "noattn", "noctx", "nomask"
VARIANT = "full"


def _emit(ctx: ExitStack, tc: tile.TileContext, xT, wqT, wkT, wvT, woT, keepT, outT):
    nc = tc.nc

    const = ctx.enter_context(tc.tile_pool(name="const", bufs=1))
    sb = ctx.enter_context(tc.tile_pool(name="sb", bufs=1))
    xtp = ctx.enter_context(tc.tile_pool(name="xtp", bufs=2))
    keepp = ctx.enter_context(tc.tile_pool(name="keepp", bufs=2))
    wp = ctx.enter_context(tc.tile_pool(name="wp", bufs=3))
    stg = ctx.enter_context(tc.tile_pool(name="stg", bufs=3))
    ps = ctx.enter_context(tc.tile_pool(name="ps", bufs=1, space="PSUM"))

    # ---- constants / weights in SBUF ----
    wq_sb = const.tile([P, NDT, FPC], F32R)
    nc.sync.dma_start(wq_sb[:], wqT[:])
    wk_sb = const.tile([P, NDT, FPC], F32R)
    nc.sync.dma_start(wk_sb[:], wkT[:])
    wv_sb = const.tile([P, NDT, FPC], F32R)
    nc.sync.dma_start(wv_sb[:], wvT[:])
    wo_sb = const.tile([P, FPC // P, D], F32R)
    nc.sync.dma_start(wo_sb[:], woT[:])
    ones_sb = const.tile([P, DH], F16)
    nc.vector.memset(ones_sb[:], 1.0)

    # ---- persistent activations ----
    q_sb = [sb.tile([P, S], F32R, name=f"q_sb{i}") for i in range(2)]
    k_sb = [sb.tile([P, S], F32R, name=f"k_sb{i}") for i in range(2)]
    v_sb = [sb.tile([P, FPC], F16, name=f"v_sb{i}") for i in range(NKT)]
    ctx_sb = [sb.tile([P, S], F32R, name=f"ctx_sb{i}") for i in range(2)]

    # ---- phase 1: projections ----
    for sc in range(NQC):
        xt_sc = xtp.tile([P, NDT, SC], F32R, tag="xt", name=f"xt_{sc}")
        nc.sync.dma_start(xt_sc[:], xT[sc])
        if VARIANT == "dma":
            continue
        for pair in range(2):
            for wi, (w_sb, dst) in enumerate(((wq_sb, q_sb), (wk_sb, k_sb))):
                mm = ps.tile([P, SC], F32, tag=("ctxX", "ctxY")[(2 * pair + wi) % 2],
                             bufs=2, name=f"qk_{sc}_{pair}_{wi}")
                for dt in range(NDT):
                    nc.tensor.matmul(
                        mm[:],
                        w_sb[:, dt, pair * P:(pair + 1) * P],
                        xt_sc[:, dt, :],
                        start=(dt == 0),
                        stop=(dt == NDT - 1),
                    )
                nc.vector.tensor_copy(dst[pair][:, sc * SC:(sc + 1) * SC], mm[:])
        for ssub in range(SC // P):
            kt = sc * (SC // P) + ssub
            vm = ps.tile([P, FPC], F32, tag=("ctxX", "ctxY")[kt % 2], bufs=2,
                         name=f"v_{kt}")
            for dt in range(NDT):
                nc.tensor.matmul(
                    vm[:],
                    xt_sc[:, dt, ssub * P:(ssub + 1) * P],
                    wv_sb[:, dt, :],
                    start=(dt == 0),
                    stop=(dt == NDT - 1),
                )
            nc.vector.tensor_copy(v_sb[kt][:], vm[:])

    # ---- phases 2+3: attention ----
    for qc in range(NQC):
        keep_sb = keepp.tile([P, NKT, SC], F16, tag="keep", name=f"keep_{qc}")
        nc.scalar.dma_start(keep_sb[:], keepT[qc])
        if VARIANT in ("dma", "noattn"):
            continue
        for pair in range(2):
            # bank Y: ctx_a on [0:64], denom_b on [64:128]
            # bank X: denom_a on [0:64], ctx_b on [64:128]
            # Two independent accumulation regions share each bank, so no
            # start/stop groups: memset to zero, then accumulate without
            # start (first PE write either overwrites (has_written clear)
            # or adds to zero (has_written stale-set) - correct either way).
            ctx_y = ps.tile([P, SC], F32, tag="ctxY", bufs=2, name=f"ctxY_{qc}_{pair}")
            ctx_x = ps.tile([P, SC], F32, tag="ctxX", bufs=2, name=f"ctxX_{qc}_{pair}")
            nc.vector.memset(ctx_y[:], 0.0)
            nc.vector.memset(ctx_x[:], 0.0)
            for kt in range(NKT):
                sc_a = ps.tile([P, SC], F32, tag="scA", bufs=2, name=f"sa_{qc}_{pair}_{kt}")
                sc_b = ps.tile([P, SC], F32, tag="scB", bufs=2, name=f"sb_{qc}_{pair}_{kt}")
                ksl = slice(kt * P, (kt + 1) * P)
                qsl = slice(qc * SC, (qc + 1) * SC)
                nc.tensor.matmul(
                    sc_a[:],
                    k_sb[pair][0:DH, ksl],
                    q_sb[pair][0:DH, qsl],
                    start=True, stop=True,
                )
                nc.tensor.matmul(
                    sc_b[:],
                    k_sb[pair][DH:P, ksl],
                    q_sb[pair][DH:P, qsl],
                    start=True, stop=True,
                    tile_position=(64, 0),
                )
                w = wp.tile([P, 2 * SC], F16, tag="w", name=f"w_{qc}_{pair}_{kt}")
                nc.scalar.activation(w[:, 0:SC], sc_a[:], EXP)
                nc.scalar.activation(w[:, SC:2 * SC], sc_b[:], EXP)
                if VARIANT != "nomask":
                    w3 = w[:].rearrange("p (h q) -> p h q", h=2)
                    kb = keep_sb[:, kt, :][:, None, :].to_broadcast((P, 2, SC))
                    eng = nc.gpsimd if kt % 3 == 2 else nc.vector
                    eng.tensor_tensor(w3, w3, kb, MULT)
                if VARIANT == "noctx":
                    continue
                vt = v_sb[kt]
                nc.tensor.matmul(
                    ctx_y[0:DH, :], vt[:, pair * P:pair * P + DH], w[:, 0:SC],
                    start=False, stop=False, skip_group_check=True,
                )
                nc.tensor.matmul(
                    ctx_x[DH:P, :], vt[:, pair * P + DH:(pair + 1) * P], w[:, SC:2 * SC],
                    start=False, stop=False, skip_group_check=True,
                    tile_position=(0, 64),
                )
                nc.tensor.matmul(
                    ctx_x[0:DH, :], ones_sb[:], w[:, 0:SC],
                    start=False, stop=False, skip_group_check=True,
                )
                nc.tensor.matmul(
                    ctx_y[DH:P, :], ones_sb[:], w[:, SC:2 * SC],
                    start=False, stop=False, skip_group_check=True,
                    tile_position=(0, 64),
                )
            recip = stg.tile([P, SC], F32, tag="recip", name=f"recip_{qc}_{pair}")
            nc.vector.reciprocal(recip[0:DH, :], ctx_x[0:DH, :])
            nc.vector.reciprocal(recip[DH:P, :], ctx_y[DH:P, :])
            qsl = slice(qc * SC, (qc + 1) * SC)
            nc.vector.tensor_tensor(
                ctx_sb[pair][0:DH, qsl], ctx_y[0:DH, :], recip[0:DH, :], MULT)
            nc.vector.tensor_tensor(
                ctx_sb[pair][DH:P, qsl], ctx_x[DH:P, :], recip[DH:P, :], MULT)

    # ---- phase 4: output projection (partial) ----
    for ft in range(D // P):
        st = stg.tile([P, NQC, SC], F32, tag="stage", bufs=2, name=f"st_{ft}")
        for sc in range(NQC):
            om = ps.tile([P, SC], F32, tag=("ctxX", "ctxY")[sc % 2], bufs=2, name=f"o_{ft}_{sc}")
            if VARIANT in ("dma",):
                nc.vector.memset(om[:], 0.0)
            else:
              for ph in range(FPC // P):
                nc.tensor.matmul(
                    om[:],
                    wo_sb[:, ph, ft * P:(ft + 1) * P],
                    ctx_sb[ph][:, sc * SC:(sc + 1) * SC],
                    start=(ph == 0),
                    stop=(ph == FPC // P - 1),
                )  # noqa
            nc.vector.tensor_copy(st[:, sc, :], om[:])
        nc.scalar.dma_start(outT[ft], st[:])


def build():
    nc = bacc.Bacc("TRN2", target_bir_lowering=False, debug=False, num_devices=NCORES)
    # all inputs pre-tiled on the host so every DMA line is contiguous
    xT = nc.dram_tensor("xT", [NQC, P, NDT, SC], F32R, kind="ExternalInput").ap()
    wqT = nc.dram_tensor("wqT", [P, NDT, FPC], F32R, kind="ExternalInput").ap()
    wkT = nc.dram_tensor("wkT", [P, NDT, FPC], F32R, kind="ExternalInput").ap()
    wvT = nc.dram_tensor("wvT", [P, NDT, FPC], F32R, kind="ExternalInput").ap()
    woT = nc.dram_tensor("woT", [P, FPC // P, D], F32R, kind="ExternalInput").ap()
    keepT = nc.dram_tensor("keepT", [NQC, P, NKT, SC], F16, kind="ExternalInput").ap()
    outT = nc.dram_tensor("outT", [D // P, P, NQC, SC], F32, kind="ExternalOutput").ap()
    with tile.TileContext(nc) as tc, ExitStack() as ctx:
        _emit(ctx, tc, xT, wqT, wkT, wvT, woT, keepT, outT)
    nc.compile()
    return nc


def make_in_maps(query, mask, Wq, Wk, Wv, Wo):
    scale = 1.0 / math.sqrt(DH)
    in_maps = []
    for b in range(B):
        # xT tiled: [NQC, P, NDT, SC]; element (sc, p, dt, s) = x[sc*SC+s, dt*P+p]
        xt = query[b].astype(np.float32).T.reshape(NDT, P, NQC, SC)
        xT = np.ascontiguousarray(xt.transpose(2, 1, 0, 3))
        # keep tiled: [NQC, P, NKT, SC]; element (qc, p, kt, q) = keep[kt*P+p, qc*SC+q]
        kp = (~mask[b]).T.astype(np.float16).reshape(NKT, P, NQC, SC)
        keepT = np.ascontiguousarray(kp.transpose(2, 1, 0, 3))
        for g in range(GROUPS):
            f0 = g * FPC
            def pack_w(wT):  # [D, FPC] -> [P, NDT, FPC]
                return np.ascontiguousarray(
                    wT.reshape(NDT, P, FPC).transpose(1, 0, 2))
            in_maps.append({
                "xT": xT,
                "wqT": pack_w((Wq[f0:f0 + FPC, :] * scale).T.astype(np.float32)),
                "wkT": pack_w(Wk[f0:f0 + FPC, :].T.astype(np.float32)),
                "wvT": pack_w(Wv[f0:f0 + FPC, :].T.astype(np.float32)),
                "woT": np.ascontiguousarray(
                    Wo[:, f0:f0 + FPC].T.astype(np.float32)
                    .reshape(FPC // P, P, D).transpose(1, 0, 2)),
                "keepT": keepT,
            })
    return in_maps


_NC_CACHE = {}


def _get_nc():
    if "nc" not in _NC_CACHE:
        _NC_CACHE["nc"] = build()
    return _NC_CACHE["nc"]


def gather(results, bo):
    out = np.empty((B, S, D), dtype=np.float32)
    for b in range(B):
        acc = results[b * GROUPS]["outT"].astype(np.float32).copy()
        for g in range(1, GROUPS):
            acc += results[b * GROUPS + g]["outT"]
        out[b] = acc.reshape(D, S).T + bo.astype(np.float32)
    return out


def kernel(query, mask, Wq, Wk, Wv, Wo, bo, **kwargs):
    nc = _get_nc()
    in_maps = make_in_maps(np.asarray(query), np.asarray(mask), np.asarray(Wq),
                           np.asarray(Wk), np.asarray(Wv), np.asarray(Wo))
    res = run_bass_kernel_spmd(nc, in_maps, list(range(NCORES)))
    return gather(res.results, np.asarray(bo))
